# revision 22
# baseline (speedup 1.0000x reference)
"""
Trainium2 Bass kernel for windowed multi-head attention with relative position
bias (Swin-style), data-parallel over the 16 windows across 8 NeuronCores.

Reference computation (per window b of 16, N=1024 tokens, C=256 channels,
H=8 heads, hd=32):
    qkv  = x @ qkv_w.T                    -> q, k, v  [B, H, N, hd]
    attn = softmax(q k^T * hd^-0.5 + bias_table[rel_pos_index])
    out  = (attn @ v)  reshaped -> [B, N, C]
    y    = out @ proj_w.T + proj_b

Device strategy (per core: 2 windows, everything in "transposed" space):
  - host: transpose x -> xT [c, n], pre-scale q-weights by hd^-0.5, gather
    exp(bias_table)[rel_pos_index] to bf16 (softmax is factorized as
    exp(s)*exp(bias), valid since scores are bounded ~|s|<12).
  - qkvT = wqkvT.T @ xT on PE; co-tiles give per-4-head stacks qT4/kT4/vT4
    with head h at partitions [32h, 32h+32) - exactly the layout needed for
    4-way tile_position packing of the K=32 score matmuls.
  - scoresT[m,n] per head via row-tiled matmuls (4 heads concurrent).
  - ACT: exp(scoresT) PSUM->SBUF bf16 (the throughput wall: 16.8M exps/core).
  - DVE: one bf16 2x tensor_tensor multiply by the gathered exp-bias.
  - attn@v and the softmax denominator both via col-tiled matmuls
    (4 heads + 4 ones-columns), accumulating over m in PSUM.
  - normalize with reciprocal + 32-partition-quadrant broadcast shuffle,
    then the output projection (+bias via a K=1 ones matmul).
"""

import functools

import ml_dtypes
import numpy as np

import concourse.bacc as bacc
import concourse.tile as tile
from concourse import mybir
from concourse.bass_utils import run_bass_kernel_spmd

BF = mybir.dt.bfloat16
F32 = mybir.dt.float32
NPBF = ml_dtypes.bfloat16

NCORES = 8
B = 16  # windows total
BPC = B // NCORES  # windows per core (2)
N = 1024  # tokens per window
C = 256  # channels
H = 8  # heads
HD = 32  # head dim
SCALE = HD**-0.5
NT = BPC * N  # tokens per core (2048)
EXPF = mybir.ActivationFunctionType.Exp


def _emit(tc):
    nc = tc.nc
    xT_d = nc.dram_tensor("xT", [128, 2, NT], BF, kind="ExternalInput")
    wq_d = nc.dram_tensor("wqkvT", [128, 2, 3 * C], BF, kind="ExternalInput")
    pw_d = nc.dram_tensor("projwT", [128, 2, C], BF, kind="ExternalInput")
    pb_d = nc.dram_tensor("pbias", [2, 128, 1], F32, kind="ExternalInput")
    eb_d = nc.dram_tensor("expb", [2, 2, 128, 8, 2048], BF, kind="ExternalInput")
    y_d = nc.dram_tensor("yT", [2, 128, NT], F32, kind="ExternalOutput")

    with (
        tc.tile_pool(name="const", bufs=1) as cp,
        tc.tile_pool(name="xp", bufs=1) as xp,
        tc.tile_pool(name="qkvp", bufs=1) as qkvp,
        tc.tile_pool(name="vp", bufs=1) as vp,
        tc.tile_pool(name="ebp", bufs=2) as ebp,
        tc.tile_pool(name="esp", bufs=6) as esp,
        tc.tile_pool(name="eap", bufs=4) as eap,
        tc.tile_pool(name="ocp", bufs=1) as ocp,
        tc.tile_pool(name="rcp", bufs=2) as rcp,
        tc.tile_pool(name="yp", bufs=3) as ysp,
    ):
        wq_sb = cp.tile([128, 2, 3 * C], BF)
        pw_sb = cp.tile([128, 2, C], BF)
        pb_sb = cp.tile([128, 2], F32)
        ones128 = cp.tile([128, 32], BF)
        xT_sb = xp.tile([128, 2, NT], BF)
        # qkv co-tiles: 0,1 = q heads 0-3/4-7 ; 2,3 = k ; 4,5 = v
        qkv_sb = qkvp.tile([128, 6, NT], BF)
        # v_aug blocks: [m % 128, b*8+mt, hg*128 + hl*32 + d]
        v_sb = vp.tile([128, 16, 256], BF)
        # out_catT: [co % 128, hg, n]  (co = (4*hg+hl)*32+d, n = b*1024+t)
        oc_sb = ocp.tile([128, 2, NT], BF)

        nc.sync.dma_start(wq_sb[:], wq_d[:])
        for kc in range(2):
            nc.sync.dma_start(xT_sb[:, kc, :], xT_d[:, kc, :])
        nc.sync.dma_start(pw_sb[:], pw_d[:])
        nc.sync.dma_start(pb_sb[:], pb_d.rearrange("ct p one -> p (ct one)"))
        nc.gpsimd.memset(ones128[:], 1.0)

        # ---- phase 1: qT/kT co-tiles via wqkvT.T @ xT; v directly in
        # [m, d] layout via xT.T @ wvT (no transposes needed). Evacuations
        # alternate DVE / ScalarE (ScalarE is idle before the exp stream).
        with (
            tc.tile_pool(name="p1", bufs=2, space="PSUM") as p1,
            tc.tile_pool(name="pv", bufs=4, space="PSUM") as pv,
        ):
            def qk_tile(ct, nch2, eng):
                pq = p1.tile([128, 1024], F32, tag="p1", name=f"pq{ct}{nch2}")
                for half in range(2):
                    for kc in range(2):
                        nc.tensor.matmul(
                            pq[:, half * 512 : (half + 1) * 512],
                            wq_sb[:, kc, ct * 128 : (ct + 1) * 128],
                            xT_sb[
                                :,
                                kc,
                                nch2 * 1024
                                + half * 512 : nch2 * 1024
                                + (half + 1) * 512,
                            ],
                            start=(kc == 0),
                            stop=(kc == 1),
                        )
                if eng == 0:
                    nc.vector.tensor_copy(
                        qkv_sb[:, ct, nch2 * 1024 : (nch2 + 1) * 1024], pq[:]
                    )
                else:
                    nc.scalar.copy(
                        qkv_sb[:, ct, nch2 * 1024 : (nch2 + 1) * 1024], pq[:]
                    )

            def v_tile(i, eng):
                pvt = pv.tile([128, 256], F32, tag="pv", name=f"pv{i}")
                for kc in range(2):
                    nc.tensor.matmul(
                        pvt[:],
                        xT_sb[:, kc, i * 128 : (i + 1) * 128],
                        wq_sb[:, kc, 2 * C : 3 * C],
                        start=(kc == 0),
                        stop=(kc == 1),
                    )
                if eng == 0:
                    nc.vector.tensor_copy(v_sb[:, i, :], pvt[:])
                else:
                    nc.scalar.copy(v_sb[:, i, :], pvt[:])

            e = 0
            for ct in (0, 2):
                for nch2 in range(2):
                    qk_tile(ct, nch2, e % 2)
                    e += 1
            for i in range(16):
                v_tile(i, i % 2)
            for ct in (1, 3):
                for nch2 in range(2):
                    qk_tile(ct, nch2, e % 2)
                    e += 1

        # ---- phase 2: attention, software-pipelined ----
        # Per step (nc2, hg, mt, b): emit the scores matmuls + exps FIRST, then
        # the previous step's bias-mul / attn@v / colsum. This keeps next-step
        # scores ahead of av/cs in the PE FIFO so the ACT engine's ring
        # (exp -> scores -> exp) never includes the accumulation matmuls.
        with (
            tc.tile_pool(name="psc", bufs=2, space="PSUM") as psc,
            tc.tile_pool(name="pav", bufs=2, space="PSUM") as pav,
            tc.tile_pool(name="pcs", bufs=1, space="PSUM") as pcs,
        ):
            steps = [
                (nc2, hg, mt, b)
                for nc2 in range(2)
                for hg in range(2)
                for mt in range(8)
                for b in range(2)
            ]
            blocks = {}  # (nc2, hg) -> dict(cs, avs, rc, slab)
            state = {}  # step -> (es, ea)

            def emit_head(step):
                nc2, hg, mt, b = step
                if (nc2, hg) not in blocks:
                    cs = pcs.tile([128, 1024], F32, tag="cs", name=f"cs{nc2}{hg}")
                    avs = [
                        pav.tile([128, 512], F32, tag="av", name=f"av{nc2}{hg}{b2}")
                        for b2 in range(2)
                    ]
                    rc = rcp.tile([128, 1024], F32, tag="rc", name=f"rc{nc2}{hg}")
                    blocks[(nc2, hg)] = dict(slab=slabs[(nc2, hg)], cs=cs, avs=avs, rc=rc)
                es = esp.tile([128, 2048], BF, tag="es", name=f"es{mt}{b}")
                for g in range(2):
                    scp = psc.tile([128, 1024], F32, tag="sc", name=f"sc{g}")
                    for j in range(2):
                        hl = 2 * g + j
                        nc.tensor.matmul(
                            scp[:, j * 512 : (j + 1) * 512],
                            qkv_sb[
                                32 * hl : 32 * hl + 32,
                                2 + hg,
                                b * N + mt * 128 : b * N + mt * 128 + 128,
                            ],
                            qkv_sb[
                                32 * hl : 32 * hl + 32,
                                hg,
                                b * N + nc2 * 512 : b * N + nc2 * 512 + 512,
                            ],
                            start=True,
                            stop=True,
                            tile_position=(32 * hl, 0),
                        )
                    nc.scalar.activation(
                        es[:, g * 1024 : (g + 1) * 1024], scp[:], EXPF
                    )
                state[step] = es

            def emit_tail(step):
                nc2, hg, mt, b = step
                blk = blocks[(nc2, hg)]
                es = state.pop(step)
                ea = eap.tile([128, 2048], BF)
                nc.vector.tensor_mul(ea[:], es[:], blk["slab"][:, mt, :])
                for hl in range(4):
                    nc.tensor.matmul(
                        blk["avs"][b][32 * hl : 32 * hl + 32, :],
                        v_sb[:, b * 8 + mt, hg * 128 + 32 * hl : hg * 128 + 32 * hl + 32],
                        ea[:, hl * 512 : (hl + 1) * 512],
                        start=(mt == 0),
                        stop=(mt == 7),
                        tile_position=(0, 32 * hl),
                        skip_group_check=True,
                    )
                    nc.tensor.matmul(
                        blk["cs"][32 * hl : 32 * hl + 32, b * 512 : (b + 1) * 512],
                        ones128[:],
                        ea[:, hl * 512 : (hl + 1) * 512],
                        start=(mt == 0),
                        stop=(mt == 7),
                        tile_position=(0, 32 * hl),
                        skip_group_check=True,
                    )
                if mt == 7:
                    # this window's colsum half is complete: reciprocal + norm
                    nc.vector.reciprocal_approx_fast(
                        out=blk["rc"][:, b * 512 : (b + 1) * 512],
                        in_=blk["cs"][:, b * 512 : (b + 1) * 512],
                    )
                    nc.vector.tensor_mul(
                        oc_sb[:, hg, b * N + nc2 * 512 : b * N + nc2 * 512 + 512],
                        blk["avs"][b][:],
                        blk["rc"][:, b * 512 : (b + 1) * 512],
                    )

            slabs = {}

            def prefetch_slab(bi):
                nc2, hg = [(n, h) for n in range(2) for h in range(2)][bi]
                slab = ebp.tile([128, 8, 2048], BF, tag="slab", name=f"slab{nc2}{hg}")
                nc.sync.dma_start(slab[:], eb_d[hg, nc2])
                slabs[(nc2, hg)] = slab

            SKEW = 2
            prefetch_slab(0)
            for i, step in enumerate(steps):
                if i % 16 == 8 and i // 16 + 1 < 4:
                    prefetch_slab(i // 16 + 1)
                emit_head(step)
                if i >= SKEW:
                    emit_tail(steps[i - SKEW])
            for j in range(SKEW, 0, -1):
                emit_tail(steps[len(steps) - j])

        # ---- phase 3: yT[c, n] = sum_co proj_w[c, co] out_cat[n, co] + pb[c] ----
        with tc.tile_pool(name="pyp", bufs=4, space="PSUM") as pyp:
            for ct in range(2):
                for nch in range(2):
                    yps = pyp.tile([128, 1024], F32)
                    for half in range(2):
                        for hg in range(2):
                            nc.tensor.matmul(
                                yps[:, half * 512 : (half + 1) * 512],
                                pw_sb[:, hg, ct * 128 : (ct + 1) * 128],
                                oc_sb[:, hg, nch * 1024 + half * 512 : nch * 1024 + (half + 1) * 512],
                                start=(hg == 0),
                                stop=(hg == 1),
                                skip_group_check=True,
                            )
                    yt = ysp.tile([128, 1024], F32)
                    nc.vector.tensor_scalar_add(yt[:], yps[:], pb_sb[:, ct : ct + 1])
                    nc.sync.dma_start(
                        y_d[ct, :, nch * 1024 : (nch + 1) * 1024], yt[:]
                    )


@functools.cache
def _build_nc():
    nc = bacc.Bacc("TRN2", target_bir_lowering=False, debug=False)
    with tile.TileContext(nc) as tc:
        _emit(tc)
    nc.compile()
    return nc


def _prep_shared(qkv_w, proj_w, proj_b, bias_table, rel_pos_index):
    w2 = np.asarray(qkv_w, np.float32).copy()
    w2[:C] *= SCALE  # fold the attention scale into the q projection
    wqkvT = np.ascontiguousarray(
        w2.T.reshape(2, 128, 3 * C).transpose(1, 0, 2)
    ).astype(NPBF)
    pwT = np.ascontiguousarray(
        np.asarray(proj_w, np.float32).T.reshape(2, 128, C).transpose(1, 0, 2)
    ).astype(NPBF)
    pb = np.ascontiguousarray(
        np.asarray(proj_b, np.float32).reshape(2, 128, 1)
    )
    E = np.exp(np.asarray(bias_table, np.float32))  # [3969, 8]
    G = E[np.asarray(rel_pos_index)]  # [n, m, 8]
    # expb[hg, nc2, mp, mt, hl, nn] = G[nc2*512+nn, mt*128+mp, 4*hg+hl]
    Gr = G.reshape(2, 512, 8, 128, 2, 4)  # [nc2, nn, mt, mp, hg, hl]
    expb = np.ascontiguousarray(Gr.transpose(4, 0, 3, 2, 5, 1)).astype(NPBF)
    expb = expb.reshape(2, 2, 128, 8, 2048)
    return wqkvT, pwT, pb, expb


def _prep_x(x, c):
    xs = np.asarray(x[c * BPC : (c + 1) * BPC], np.float32)  # [2, 1024, 256]
    a = xs.transpose(2, 0, 1).reshape(C, NT)  # [c_in, b*1024+t]
    return np.ascontiguousarray(a.reshape(2, 128, NT).transpose(1, 0, 2)).astype(NPBF)


def _in_maps(x, qkv_w, proj_w, proj_b, bias_table, rel_pos_index):
    wqkvT, pwT, pb, expb = _prep_shared(
        qkv_w, proj_w, proj_b, bias_table, rel_pos_index
    )
    return [
        {
            "xT": _prep_x(x, c),
            "wqkvT": wqkvT,
            "projwT": pwT,
            "pbias": pb,
            "expb": expb,
        }
        for c in range(NCORES)
    ]


def run(x, qkv_w, proj_w, proj_b, bias_table, rel_pos_index, **run_kwargs):
    nc = _build_nc()
    in_maps = _in_maps(x, qkv_w, proj_w, proj_b, bias_table, rel_pos_index)
    res = run_bass_kernel_spmd(nc, in_maps, list(range(NCORES)), **run_kwargs)
    y = np.stack(
        [
            res.results[c]["yT"].reshape(C, NT).reshape(C, BPC, N).transpose(1, 2, 0)
            for c in range(NCORES)
        ]
    )
    return y.reshape(B, N, C).astype(np.float32), res


def kernel(x, qkv_w, proj_w, proj_b, bias_table, rel_pos_index):
    y, _ = run(x, qkv_w, proj_w, proj_b, bias_table, rel_pos_index)
    return y


# revision 23
# speedup vs baseline: 1.0664x; 1.0664x over previous
"""
Trainium2 Bass kernel for windowed multi-head attention with relative position
bias (Swin-style), data-parallel over the 16 windows across 8 NeuronCores.

Reference computation (per window b of 16, N=1024 tokens, C=256 channels,
H=8 heads, hd=32):
    qkv  = x @ qkv_w.T                    -> q, k, v  [B, H, N, hd]
    attn = softmax(q k^T * hd^-0.5 + bias_table[rel_pos_index])
    out  = (attn @ v)  reshaped -> [B, N, C]
    y    = out @ proj_w.T + proj_b

Device strategy (per core: 2 windows, everything in "transposed" space):
  - host: transpose x -> xT [c, n], pre-scale q-weights by hd^-0.5, gather
    exp(bias_table)[rel_pos_index] to bf16 (softmax is factorized as
    exp(s)*exp(bias), valid since scores are bounded ~|s|<12).
  - qkvT = wqkvT.T @ xT on PE; co-tiles give per-4-head stacks qT4/kT4/vT4
    with head h at partitions [32h, 32h+32) - exactly the layout needed for
    4-way tile_position packing of the K=32 score matmuls.
  - scoresT[m,n] per head via row-tiled matmuls (4 heads concurrent).
  - ACT: exp(scoresT) PSUM->SBUF bf16 (the throughput wall: 16.8M exps/core).
  - DVE: one bf16 2x tensor_tensor multiply by the gathered exp-bias.
  - attn@v and the softmax denominator both via col-tiled matmuls
    (4 heads + 4 ones-columns), accumulating over m in PSUM.
  - normalize with reciprocal + 32-partition-quadrant broadcast shuffle,
    then the output projection (+bias via a K=1 ones matmul).
"""

import functools

import ml_dtypes
import numpy as np

import concourse.bacc as bacc
import concourse.tile as tile
from concourse import mybir
from concourse.bass_utils import run_bass_kernel_spmd

BF = mybir.dt.bfloat16
F32 = mybir.dt.float32
NPBF = ml_dtypes.bfloat16

NCORES = 8
B = 16  # windows total
BPC = B // NCORES  # windows per core (2)
N = 1024  # tokens per window
C = 256  # channels
H = 8  # heads
HD = 32  # head dim
SCALE = HD**-0.5
NT = BPC * N  # tokens per core (2048)
EXPF = mybir.ActivationFunctionType.Exp


def _emit(tc):
    nc = tc.nc
    xT_d = nc.dram_tensor("xT", [128, 2, NT], BF, kind="ExternalInput")
    wq_d = nc.dram_tensor("wqkvT", [128, 2, 3 * C], BF, kind="ExternalInput")
    pw_d = nc.dram_tensor("projwT", [128, 2, C], BF, kind="ExternalInput")
    pb_d = nc.dram_tensor("pbias", [2, 128, 1], F32, kind="ExternalInput")
    eb_d = nc.dram_tensor("expb", [2, 2, 128, 8, 2048], BF, kind="ExternalInput")
    y_d = nc.dram_tensor("yT", [2, 128, NT], F32, kind="ExternalOutput")

    with (
        tc.tile_pool(name="const", bufs=1) as cp,
        tc.tile_pool(name="xp", bufs=1) as xp,
        tc.tile_pool(name="qkvp", bufs=1) as qkvp,
        tc.tile_pool(name="vp", bufs=1) as vp,
        tc.tile_pool(name="ebp", bufs=2) as ebp,
        tc.tile_pool(name="esp", bufs=6) as esp,
        tc.tile_pool(name="eap", bufs=4) as eap,
        tc.tile_pool(name="ocp", bufs=1) as ocp,
        tc.tile_pool(name="rcp", bufs=2) as rcp,
        tc.tile_pool(name="yp", bufs=3) as ysp,
    ):
        wq_sb = cp.tile([128, 2, 3 * C], BF)
        pw_sb = cp.tile([128, 2, C], BF)
        pb_sb = cp.tile([128, 2], F32)
        ones128 = cp.tile([128, 32], BF)
        xT_sb = xp.tile([128, 2, NT], BF)
        # qkv co-tiles: 0,1 = q heads 0-3/4-7 ; 2,3 = k ; 4,5 = v
        qkv_sb = qkvp.tile([128, 6, NT], BF)
        # v_aug blocks: [m % 128, b*8+mt, hg*128 + hl*32 + d]
        v_sb = vp.tile([128, 16, 256], BF)
        # out_catT: [co % 128, hg, n]  (co = (4*hg+hl)*32+d, n = b*1024+t)
        oc_sb = ocp.tile([128, 2, NT], BF)

        nc.sync.dma_start(wq_sb[:], wq_d[:])
        for kc in range(2):
            nc.sync.dma_start(xT_sb[:, kc, :], xT_d[:, kc, :])
        nc.sync.dma_start(pw_sb[:], pw_d[:])
        nc.sync.dma_start(pb_sb[:], pb_d.rearrange("ct p one -> p (ct one)"))
        nc.gpsimd.memset(ones128[:], 1.0)

        # ---- phase 1: qT/kT co-tiles via wqkvT.T @ xT; v directly in
        # [m, d] layout via xT.T @ wvT (no transposes needed). Evacuations
        # alternate DVE / ScalarE (ScalarE is idle before the exp stream).
        with (
            tc.tile_pool(name="p1", bufs=2, space="PSUM") as p1,
            tc.tile_pool(name="pv", bufs=4, space="PSUM") as pv,
        ):
            def qk_tile(ct, nch2, eng):
                pq = p1.tile([128, 1024], F32, tag="p1", name=f"pq{ct}{nch2}")
                for half in range(2):
                    for kc in range(2):
                        nc.tensor.matmul(
                            pq[:, half * 512 : (half + 1) * 512],
                            wq_sb[:, kc, ct * 128 : (ct + 1) * 128],
                            xT_sb[
                                :,
                                kc,
                                nch2 * 1024
                                + half * 512 : nch2 * 1024
                                + (half + 1) * 512,
                            ],
                            start=(kc == 0),
                            stop=(kc == 1),
                        )
                if eng == 0:
                    nc.vector.tensor_copy(
                        qkv_sb[:, ct, nch2 * 1024 : (nch2 + 1) * 1024], pq[:]
                    )
                else:
                    nc.scalar.copy(
                        qkv_sb[:, ct, nch2 * 1024 : (nch2 + 1) * 1024], pq[:]
                    )

            def v_tile(i, eng):
                pvt = pv.tile([128, 256], F32, tag="pv", name=f"pv{i}")
                for kc in range(2):
                    nc.tensor.matmul(
                        pvt[:],
                        xT_sb[:, kc, i * 128 : (i + 1) * 128],
                        wq_sb[:, kc, 2 * C : 3 * C],
                        start=(kc == 0),
                        stop=(kc == 1),
                    )
                if eng == 0:
                    nc.vector.tensor_copy(v_sb[:, i, :], pvt[:])
                else:
                    nc.scalar.copy(v_sb[:, i, :], pvt[:])

            e = 0
            for ct in (0, 2):
                for nch2 in range(2):
                    qk_tile(ct, nch2, e % 2)
                    e += 1
            for i in range(16):
                v_tile(i, i % 2)
            for ct in (1, 3):
                for nch2 in range(2):
                    qk_tile(ct, nch2, e % 2)
                    e += 1

        # ---- phase 2: attention, software-pipelined ----
        # Per step (nc2, hg, mt, b): emit the scores matmuls + exps FIRST, then
        # the previous step's bias-mul / attn@v / colsum. This keeps next-step
        # scores ahead of av/cs in the PE FIFO so the ACT engine's ring
        # (exp -> scores -> exp) never includes the accumulation matmuls.
        with (
            tc.tile_pool(name="psc", bufs=3, space="PSUM") as psc,
            tc.tile_pool(name="pav", bufs=1, space="PSUM") as pav,
            tc.tile_pool(name="pcs", bufs=1, space="PSUM") as pcs,
        ):
            steps = [
                (nc2, hg, mt, b)
                for nc2 in range(2)
                for hg in range(2)
                for b in range(2)
                for mt in range(8)
            ]
            blocks = {}  # (nc2, hg) -> dict(cs, avs, rc, slab)
            state = {}  # step -> (es, ea)

            def emit_head(step):
                nc2, hg, mt, b = step
                if (nc2, hg, b) not in blocks:
                    cs = pcs.tile([128, 512], F32, tag="cs", name=f"cs{nc2}{hg}{b}")
                    av = pav.tile([128, 512], F32, tag="av", name=f"av{nc2}{hg}{b}")
                    rc = rcp.tile([128, 512], F32, tag="rc", name=f"rc{nc2}{hg}{b}")
                    blocks[(nc2, hg, b)] = dict(
                        slab=slabs[(nc2, hg)], cs=cs, av=av, rc=rc
                    )
                es = esp.tile([128, 2048], BF, tag="es", name=f"es{mt}{b}")
                for g in range(2):
                    scp = psc.tile([128, 1024], F32, tag="sc", name=f"sc{g}")
                    for j in range(2):
                        hl = 2 * g + j
                        nc.tensor.matmul(
                            scp[:, j * 512 : (j + 1) * 512],
                            qkv_sb[
                                32 * hl : 32 * hl + 32,
                                2 + hg,
                                b * N + mt * 128 : b * N + mt * 128 + 128,
                            ],
                            qkv_sb[
                                32 * hl : 32 * hl + 32,
                                hg,
                                b * N + nc2 * 512 : b * N + nc2 * 512 + 512,
                            ],
                            start=True,
                            stop=True,
                            tile_position=(32 * hl, 0),
                        )
                    nc.scalar.activation(
                        es[:, g * 1024 : (g + 1) * 1024], scp[:], EXPF
                    )
                state[step] = es

            def emit_tail(step):
                nc2, hg, mt, b = step
                blk = blocks[(nc2, hg, b)]
                es = state.pop(step)
                ea = eap.tile([128, 2048], BF)
                nc.vector.tensor_mul(ea[:], es[:], blk["slab"][:, mt, :])
                for hl in range(4):
                    nc.tensor.matmul(
                        blk["av"][32 * hl : 32 * hl + 32, :],
                        v_sb[:, b * 8 + mt, hg * 128 + 32 * hl : hg * 128 + 32 * hl + 32],
                        ea[:, hl * 512 : (hl + 1) * 512],
                        start=(mt == 0),
                        stop=(mt == 7),
                        tile_position=(0, 32 * hl),
                        skip_group_check=True,
                    )
                    nc.tensor.matmul(
                        blk["cs"][32 * hl : 32 * hl + 32, :],
                        ones128[:],
                        ea[:, hl * 512 : (hl + 1) * 512],
                        start=(mt == 0),
                        stop=(mt == 7),
                        tile_position=(0, 32 * hl),
                        skip_group_check=True,
                    )
                if mt == 7:
                    # this window's colsum is complete: reciprocal + normalize
                    nc.vector.reciprocal_approx_fast(out=blk["rc"][:], in_=blk["cs"][:])
                    nc.vector.tensor_mul(
                        oc_sb[:, hg, b * N + nc2 * 512 : b * N + nc2 * 512 + 512],
                        blk["av"][:],
                        blk["rc"][:],
                    )

            slabs = {}

            def prefetch_slab(bi):
                nc2, hg = [(n, h) for n in range(2) for h in range(2)][bi]
                slab = ebp.tile([128, 8, 2048], BF, tag="slab", name=f"slab{nc2}{hg}")
                nc.sync.dma_start(slab[:], eb_d[hg, nc2])
                slabs[(nc2, hg)] = slab

            SKEW = 2
            prefetch_slab(0)
            for i, step in enumerate(steps):
                if i % 16 == 8 and i // 16 + 1 < 4:
                    prefetch_slab(i // 16 + 1)
                emit_head(step)
                if i >= SKEW:
                    emit_tail(steps[i - SKEW])
            for j in range(SKEW, 0, -1):
                emit_tail(steps[len(steps) - j])

        # ---- phase 3: yT[c, n] = sum_co proj_w[c, co] out_cat[n, co] + pb[c] ----
        with tc.tile_pool(name="pyp", bufs=4, space="PSUM") as pyp:
            for ct in range(2):
                for nch in range(2):
                    yps = pyp.tile([128, 1024], F32)
                    for half in range(2):
                        for hg in range(2):
                            nc.tensor.matmul(
                                yps[:, half * 512 : (half + 1) * 512],
                                pw_sb[:, hg, ct * 128 : (ct + 1) * 128],
                                oc_sb[:, hg, nch * 1024 + half * 512 : nch * 1024 + (half + 1) * 512],
                                start=(hg == 0),
                                stop=(hg == 1),
                                skip_group_check=True,
                            )
                    yt = ysp.tile([128, 1024], F32)
                    nc.vector.tensor_scalar_add(yt[:], yps[:], pb_sb[:, ct : ct + 1])
                    nc.sync.dma_start(
                        y_d[ct, :, nch * 1024 : (nch + 1) * 1024], yt[:]
                    )


@functools.cache
def _build_nc():
    nc = bacc.Bacc("TRN2", target_bir_lowering=False, debug=False)
    with tile.TileContext(nc) as tc:
        _emit(tc)
    nc.compile()
    return nc


def _prep_shared(qkv_w, proj_w, proj_b, bias_table, rel_pos_index):
    w2 = np.asarray(qkv_w, np.float32).copy()
    w2[:C] *= SCALE  # fold the attention scale into the q projection
    wqkvT = np.ascontiguousarray(
        w2.T.reshape(2, 128, 3 * C).transpose(1, 0, 2)
    ).astype(NPBF)
    pwT = np.ascontiguousarray(
        np.asarray(proj_w, np.float32).T.reshape(2, 128, C).transpose(1, 0, 2)
    ).astype(NPBF)
    pb = np.ascontiguousarray(
        np.asarray(proj_b, np.float32).reshape(2, 128, 1)
    )
    E = np.exp(np.asarray(bias_table, np.float32))  # [3969, 8]
    G = E[np.asarray(rel_pos_index)]  # [n, m, 8]
    # expb[hg, nc2, mp, mt, hl, nn] = G[nc2*512+nn, mt*128+mp, 4*hg+hl]
    Gr = G.reshape(2, 512, 8, 128, 2, 4)  # [nc2, nn, mt, mp, hg, hl]
    expb = np.ascontiguousarray(Gr.transpose(4, 0, 3, 2, 5, 1)).astype(NPBF)
    expb = expb.reshape(2, 2, 128, 8, 2048)
    return wqkvT, pwT, pb, expb


def _prep_x(x, c):
    xs = np.asarray(x[c * BPC : (c + 1) * BPC], np.float32)  # [2, 1024, 256]
    a = xs.transpose(2, 0, 1).reshape(C, NT)  # [c_in, b*1024+t]
    return np.ascontiguousarray(a.reshape(2, 128, NT).transpose(1, 0, 2)).astype(NPBF)


def _in_maps(x, qkv_w, proj_w, proj_b, bias_table, rel_pos_index):
    wqkvT, pwT, pb, expb = _prep_shared(
        qkv_w, proj_w, proj_b, bias_table, rel_pos_index
    )
    return [
        {
            "xT": _prep_x(x, c),
            "wqkvT": wqkvT,
            "projwT": pwT,
            "pbias": pb,
            "expb": expb,
        }
        for c in range(NCORES)
    ]


def run(x, qkv_w, proj_w, proj_b, bias_table, rel_pos_index, **run_kwargs):
    nc = _build_nc()
    in_maps = _in_maps(x, qkv_w, proj_w, proj_b, bias_table, rel_pos_index)
    res = run_bass_kernel_spmd(nc, in_maps, list(range(NCORES)), **run_kwargs)
    y = np.stack(
        [
            res.results[c]["yT"].reshape(C, NT).reshape(C, BPC, N).transpose(1, 2, 0)
            for c in range(NCORES)
        ]
    )
    return y.reshape(B, N, C).astype(np.float32), res


def kernel(x, qkv_w, proj_w, proj_b, bias_table, rel_pos_index):
    y, _ = run(x, qkv_w, proj_w, proj_b, bias_table, rel_pos_index)
    return y


# revision 26
# speedup vs baseline: 1.0722x; 1.0055x over previous
"""
Trainium2 Bass kernel for windowed multi-head attention with relative position
bias (Swin-style), data-parallel over the 16 windows across 8 NeuronCores.

Reference computation (per window b of 16, N=1024 tokens, C=256 channels,
H=8 heads, hd=32):
    qkv  = x @ qkv_w.T                    -> q, k, v  [B, H, N, hd]
    attn = softmax(q k^T * hd^-0.5 + bias_table[rel_pos_index])
    out  = (attn @ v)  reshaped -> [B, N, C]
    y    = out @ proj_w.T + proj_b

Device strategy (per core: 2 windows, everything in "transposed" space):
  - host: transpose x -> xT [c, n], pre-scale q-weights by hd^-0.5, gather
    exp(bias_table)[rel_pos_index] to bf16 (softmax is factorized as
    exp(s)*exp(bias), valid since scores are bounded ~|s|<12).
  - qT/kT co-tiles via wqkvT.T @ xT (head h at partitions [32h, 32h+32),
    the layout needed for 4-way tile_position packing of K=32 score
    matmuls); v computed directly in [token, dim] layout via xT.T @ wvT
    (no on-chip transposes anywhere).
  - scoresT[m,n] via row-tiled matmuls; ACT exp(scoresT) PSUM->SBUF bf16
    is the throughput wall (16.8M exps/core, ~1.1us per FD-1024 call);
    DVE does one bf16 2x multiply by the gathered exp-bias per tile.
  - attn@v and the softmax denominator via col-tiled matmuls (4 heads /
    4 ones-stationaries whose M=32 broadcasts each colsum across its
    32-partition group), accumulating over m in PSUM.
  - Software-pipelined emission (scores+exp of step i before mul/av/cs of
    step i-2) keeps the per-engine FIFOs from putting accumulation work in
    the exp ring; PSUM: 3-slot score ring (6 banks) + av + cs (1 each).
  - normalize with reciprocal_approx_fast; projection computed transposed
    (yT = pwT.T @ out_catT, N=512 matmuls) and untransposed on host.
"""

import functools

import ml_dtypes
import numpy as np

import concourse.bacc as bacc
import concourse.tile as tile
from concourse import mybir
from concourse.bass_utils import run_bass_kernel_spmd

BF = mybir.dt.bfloat16
F32 = mybir.dt.float32
NPBF = ml_dtypes.bfloat16

NCORES = 8
B = 16  # windows total
BPC = B // NCORES  # windows per core (2)
N = 1024  # tokens per window
C = 256  # channels
H = 8  # heads
HD = 32  # head dim
SCALE = HD**-0.5
NT = BPC * N  # tokens per core (2048)
EXPF = mybir.ActivationFunctionType.Exp


def _emit(tc):
    nc = tc.nc
    xT_d = nc.dram_tensor("xT", [128, 2, NT], BF, kind="ExternalInput")
    wq_d = nc.dram_tensor("wqkvT", [128, 2, 3 * C], BF, kind="ExternalInput")
    pw_d = nc.dram_tensor("projwT", [128, 2, C], BF, kind="ExternalInput")
    pb_d = nc.dram_tensor("pbias", [2, 128, 1], F32, kind="ExternalInput")
    eb_d = nc.dram_tensor("expb", [2, 2, 128, 8, 2048], BF, kind="ExternalInput")
    y_d = nc.dram_tensor("yT", [2, 128, NT], F32, kind="ExternalOutput")

    with (
        tc.tile_pool(name="const", bufs=1) as cp,
        tc.tile_pool(name="xp", bufs=1) as xp,
        tc.tile_pool(name="qkvp", bufs=1) as qkvp,
        tc.tile_pool(name="vp", bufs=1) as vp,
        tc.tile_pool(name="ebp", bufs=2) as ebp,
        tc.tile_pool(name="esp", bufs=6) as esp,
        tc.tile_pool(name="eap", bufs=4) as eap,
        tc.tile_pool(name="ocp", bufs=1) as ocp,
        tc.tile_pool(name="rcp", bufs=2) as rcp,
        tc.tile_pool(name="yp", bufs=3) as ysp,
    ):
        wq_sb = cp.tile([128, 2, 3 * C], BF)
        pw_sb = cp.tile([128, 2, C], BF)
        pb_sb = cp.tile([128, 2], F32)
        ones128 = cp.tile([128, 32], BF)
        xT_sb = xp.tile([128, 2, NT], BF)
        # qkv co-tiles: 0,1 = q heads 0-3/4-7 ; 2,3 = k ; 4,5 = v
        qkv_sb = qkvp.tile([128, 6, NT], BF)
        # v_aug blocks: [m % 128, b*8+mt, hg*128 + hl*32 + d]
        v_sb = vp.tile([128, 16, 256], BF)
        # out_catT: [co % 128, hg, n]  (co = (4*hg+hl)*32+d, n = b*1024+t)
        oc_sb = ocp.tile([128, 2, NT], BF)

        nc.sync.dma_start(wq_sb[:], wq_d[:])
        for kc in range(2):
            nc.sync.dma_start(xT_sb[:, kc, :], xT_d[:, kc, :])
        nc.sync.dma_start(pw_sb[:], pw_d[:])
        nc.sync.dma_start(pb_sb[:], pb_d.rearrange("ct p one -> p (ct one)"))
        nc.gpsimd.memset(ones128[:], 1.0)

        # ---- phase 1: qT/kT co-tiles via wqkvT.T @ xT; v directly in
        # [m, d] layout via xT.T @ wvT (no transposes needed). Evacuations
        # alternate DVE / ScalarE (ScalarE is idle before the exp stream).
        with (
            tc.tile_pool(name="p1", bufs=2, space="PSUM") as p1,
            tc.tile_pool(name="pv", bufs=4, space="PSUM") as pv,
        ):
            def qk_tile(ct, nch2, eng):
                pq = p1.tile([128, 1024], F32, tag="p1", name=f"pq{ct}{nch2}")
                for half in range(2):
                    for kc in range(2):
                        nc.tensor.matmul(
                            pq[:, half * 512 : (half + 1) * 512],
                            wq_sb[:, kc, ct * 128 : (ct + 1) * 128],
                            xT_sb[
                                :,
                                kc,
                                nch2 * 1024
                                + half * 512 : nch2 * 1024
                                + (half + 1) * 512,
                            ],
                            start=(kc == 0),
                            stop=(kc == 1),
                        )
                if eng == 0:
                    nc.vector.tensor_copy(
                        qkv_sb[:, ct, nch2 * 1024 : (nch2 + 1) * 1024], pq[:]
                    )
                else:
                    nc.scalar.copy(
                        qkv_sb[:, ct, nch2 * 1024 : (nch2 + 1) * 1024], pq[:]
                    )

            def v_tile(i, eng):
                pvt = pv.tile([128, 256], F32, tag="pv", name=f"pv{i}")
                for kc in range(2):
                    nc.tensor.matmul(
                        pvt[:],
                        xT_sb[:, kc, i * 128 : (i + 1) * 128],
                        wq_sb[:, kc, 2 * C : 3 * C],
                        start=(kc == 0),
                        stop=(kc == 1),
                    )
                if eng == 0:
                    nc.vector.tensor_copy(v_sb[:, i, :], pvt[:])
                else:
                    nc.scalar.copy(v_sb[:, i, :], pvt[:])

            e = 0
            for ct in (0, 2):
                for nch2 in range(2):
                    qk_tile(ct, nch2, e % 2)
                    e += 1
            for i in range(16):
                v_tile(i, i % 2)
            for ct in (1, 3):
                for nch2 in range(2):
                    qk_tile(ct, nch2, e % 2)
                    e += 1

        # ---- phase 2: attention, software-pipelined ----
        # Per step (nc2, hg, mt, b): emit the scores matmuls + exps FIRST, then
        # the previous step's bias-mul / attn@v / colsum. This keeps next-step
        # scores ahead of av/cs in the PE FIFO so the ACT engine's ring
        # (exp -> scores -> exp) never includes the accumulation matmuls.
        with (
            tc.tile_pool(name="psc", bufs=3, space="PSUM") as psc,
            tc.tile_pool(name="pav", bufs=1, space="PSUM") as pav,
            tc.tile_pool(name="pcs", bufs=1, space="PSUM") as pcs,
        ):
            steps = [
                (nc2, hg, mt, b)
                for nc2 in range(2)
                for hg in range(2)
                for b in range(2)
                for mt in range(8)
            ]
            blocks = {}  # (nc2, hg) -> dict(cs, avs, rc, slab)
            state = {}  # step -> (es, ea)

            def emit_head(step):
                nc2, hg, mt, b = step
                if (nc2, hg, b) not in blocks:
                    cs = pcs.tile([128, 512], F32, tag="cs", name=f"cs{nc2}{hg}{b}")
                    av = pav.tile([128, 512], F32, tag="av", name=f"av{nc2}{hg}{b}")
                    rc = rcp.tile([128, 512], F32, tag="rc", name=f"rc{nc2}{hg}{b}")
                    blocks[(nc2, hg, b)] = dict(
                        slab=slabs[(nc2, hg)], cs=cs, av=av, rc=rc
                    )
                es = esp.tile([128, 2048], BF, tag="es", name=f"es{mt}{b}")
                for g in range(2):
                    scp = psc.tile([128, 1024], F32, tag="sc", name=f"sc{g}")
                    for j in range(2):
                        hl = 2 * g + j
                        nc.tensor.matmul(
                            scp[:, j * 512 : (j + 1) * 512],
                            qkv_sb[
                                32 * hl : 32 * hl + 32,
                                2 + hg,
                                b * N + mt * 128 : b * N + mt * 128 + 128,
                            ],
                            qkv_sb[
                                32 * hl : 32 * hl + 32,
                                hg,
                                b * N + nc2 * 512 : b * N + nc2 * 512 + 512,
                            ],
                            start=True,
                            stop=True,
                            tile_position=(32 * hl, 0),
                        )
                    nc.scalar.activation(
                        es[:, g * 1024 : (g + 1) * 1024], scp[:], EXPF
                    )
                state[step] = es

            def emit_tail(step):
                nc2, hg, mt, b = step
                blk = blocks[(nc2, hg, b)]
                es = state.pop(step)
                ea = eap.tile([128, 2048], BF)
                nc.vector.tensor_mul(ea[:], es[:], blk["slab"][:, mt, :])
                for hl in range(4):
                    nc.tensor.matmul(
                        blk["av"][32 * hl : 32 * hl + 32, :],
                        v_sb[:, b * 8 + mt, hg * 128 + 32 * hl : hg * 128 + 32 * hl + 32],
                        ea[:, hl * 512 : (hl + 1) * 512],
                        start=(mt == 0),
                        stop=(mt == 7),
                        tile_position=(0, 32 * hl),
                        skip_group_check=True,
                    )
                    nc.tensor.matmul(
                        blk["cs"][32 * hl : 32 * hl + 32, :],
                        ones128[:],
                        ea[:, hl * 512 : (hl + 1) * 512],
                        start=(mt == 0),
                        stop=(mt == 7),
                        tile_position=(0, 32 * hl),
                        skip_group_check=True,
                    )
                if mt == 7:
                    # this window's colsum is complete: reciprocal + normalize
                    nc.vector.reciprocal_approx_fast(out=blk["rc"][:], in_=blk["cs"][:])
                    nc.vector.tensor_mul(
                        oc_sb[:, hg, b * N + nc2 * 512 : b * N + nc2 * 512 + 512],
                        blk["av"][:],
                        blk["rc"][:],
                    )

            slabs = {}

            def prefetch_slab(bi):
                nc2, hg = [(n, h) for n in range(2) for h in range(2)][bi]
                slab = ebp.tile([128, 8, 2048], BF, tag="slab", name=f"slab{nc2}{hg}")
                nc.sync.dma_start(slab[:], eb_d[hg, nc2])
                slabs[(nc2, hg)] = slab

            SKEW = 2
            prefetch_slab(0)
            for i, step in enumerate(steps):
                if i % 16 == 8 and i // 16 + 1 < 4:
                    prefetch_slab(i // 16 + 1)
                emit_head(step)
                if i >= SKEW:
                    emit_tail(steps[i - SKEW])
            for j in range(SKEW, 0, -1):
                emit_tail(steps[len(steps) - j])

        # ---- phase 3: yT[c, n] = sum_co proj_w[c, co] out_cat[n, co] + pb[c] ----
        with tc.tile_pool(name="pyp", bufs=4, space="PSUM") as pyp:
            for ct in range(2):
                for nch in range(2):
                    yps = pyp.tile([128, 1024], F32)
                    for half in range(2):
                        for hg in range(2):
                            nc.tensor.matmul(
                                yps[:, half * 512 : (half + 1) * 512],
                                pw_sb[:, hg, ct * 128 : (ct + 1) * 128],
                                oc_sb[:, hg, nch * 1024 + half * 512 : nch * 1024 + (half + 1) * 512],
                                start=(hg == 0),
                                stop=(hg == 1),
                                skip_group_check=True,
                            )
                    yt = ysp.tile([128, 1024], F32)
                    nc.vector.tensor_scalar_add(yt[:], yps[:], pb_sb[:, ct : ct + 1])
                    nc.sync.dma_start(
                        y_d[ct, :, nch * 1024 : (nch + 1) * 1024], yt[:]
                    )


@functools.cache
def _build_nc():
    nc = bacc.Bacc("TRN2", target_bir_lowering=False, debug=False)
    with tile.TileContext(nc) as tc:
        _emit(tc)
    nc.compile()
    return nc


def _prep_shared(qkv_w, proj_w, proj_b, bias_table, rel_pos_index):
    w2 = np.asarray(qkv_w, np.float32).copy()
    w2[:C] *= SCALE  # fold the attention scale into the q projection
    wqkvT = np.ascontiguousarray(
        w2.T.reshape(2, 128, 3 * C).transpose(1, 0, 2)
    ).astype(NPBF)
    pwT = np.ascontiguousarray(
        np.asarray(proj_w, np.float32).T.reshape(2, 128, C).transpose(1, 0, 2)
    ).astype(NPBF)
    pb = np.ascontiguousarray(
        np.asarray(proj_b, np.float32).reshape(2, 128, 1)
    )
    E = np.exp(np.asarray(bias_table, np.float32))  # [3969, 8]
    G = E[np.asarray(rel_pos_index)]  # [n, m, 8]
    # expb[hg, nc2, mp, mt, hl, nn] = G[nc2*512+nn, mt*128+mp, 4*hg+hl]
    Gr = G.reshape(2, 512, 8, 128, 2, 4)  # [nc2, nn, mt, mp, hg, hl]
    expb = np.ascontiguousarray(Gr.transpose(4, 0, 3, 2, 5, 1)).astype(NPBF)
    expb = expb.reshape(2, 2, 128, 8, 2048)
    return wqkvT, pwT, pb, expb


def _prep_x(x, c):
    xs = np.asarray(x[c * BPC : (c + 1) * BPC], np.float32)  # [2, 1024, 256]
    a = xs.transpose(2, 0, 1).reshape(C, NT)  # [c_in, b*1024+t]
    return np.ascontiguousarray(a.reshape(2, 128, NT).transpose(1, 0, 2)).astype(NPBF)


def _in_maps(x, qkv_w, proj_w, proj_b, bias_table, rel_pos_index):
    wqkvT, pwT, pb, expb = _prep_shared(
        qkv_w, proj_w, proj_b, bias_table, rel_pos_index
    )
    return [
        {
            "xT": _prep_x(x, c),
            "wqkvT": wqkvT,
            "projwT": pwT,
            "pbias": pb,
            "expb": expb,
        }
        for c in range(NCORES)
    ]


def run(x, qkv_w, proj_w, proj_b, bias_table, rel_pos_index, **run_kwargs):
    nc = _build_nc()
    in_maps = _in_maps(x, qkv_w, proj_w, proj_b, bias_table, rel_pos_index)
    res = run_bass_kernel_spmd(nc, in_maps, list(range(NCORES)), **run_kwargs)
    y = np.stack(
        [
            res.results[c]["yT"].reshape(C, NT).reshape(C, BPC, N).transpose(1, 2, 0)
            for c in range(NCORES)
        ]
    )
    return y.reshape(B, N, C).astype(np.float32), res


def kernel(x, qkv_w, proj_w, proj_b, bias_table, rel_pos_index):
    y, _ = run(x, qkv_w, proj_w, proj_b, bias_table, rel_pos_index)
    return y
